# revision 21
# baseline (speedup 1.0000x reference)
"""GCN (7-layer, PyG-style symmetric-normalized message passing) on 8 TRN2
NeuronCores via Bass/Tile.

Strategy (graph/data parallel over nodes):
  - Nodes are assigned to 8 cores x 49 blocks of 128 slots each (load-balanced
    by in-degree so per-block message counts are nearly equal; blocks relabeled
    within each AllGather zone so block j has similar counts on every core).
  - Gather cost model (measured): SWDGE descriptor generation serializes on
    the GpSimd engine at ~3ns/index + ~133ns/call; the 4 queues only overlap
    the SDMA drain.  The desc-gen floor (~75k idx/core/layer) is the kernel's
    critical resource, so everything else is arranged to keep it 100% busy:
      * num_idxs_reg is loaded per call from a per-core count table, so the
        engine only generates descriptors for actual edges (the static chunk
        layout is the max over cores; tail chunks keep stale data which the
        seg=-1 padding nullifies in the one-hot matmul).
      * The Z'' table AllGather is split into 3 zones (blocks 0..15, 16..31,
        32..48 of every core).  Zone AGs fire as soon as the last block of the
        zone finishes its stage A (~0.45L / ~0.75L / L into layer l), so zones
        0/1 are gathered stall-free at the start of layer l+1 and zone 2's
        calls are deferred DZ block positions to cover its AG tail.
      * Gather calls use zone-scoped table extents so Tile only makes each
        call wait for its own zone's AG.
  - dis[dst] scaling is deferred algebraically: relu(dis*x) = dis*relu(x)
    (dis>0 always, self-loops), so stage B stores the unscaled u = relu(O)
    straight from PSUM (no DVE multiply) and stage A scales by dis^2
    (dis^1 for layer 0, which consumes x).  The final readout row is scaled
    by dis before adding lin_b.
  - Per layer, per block (fused loop): 3 gather calls (one per zone; int16
    idx relative to the zone base), build one-hot selector S on DVE
    (S[msg, slot] = (iota == segid)), segment-sum on PE:
    O[feat, slot] += M_chunk.T @ S_chunk with the self-loop chunk done as
    zbuf_block.T @ I, then u' = relu(O) on ACT (PSUM read direct);
    stage A (layer l+1): Z'' = dis^2 * (u' @ W) into zbuf.
  - Final: out = dis * (lin_w.T @ u7) + lin_b, one row per core, host
    reassembles.

All index/normalization prep is host-side numpy (graph routing), baked into
per-core input tensors; the float pipeline runs on device.
"""

import math
import os
import sys
from dataclasses import dataclass

import numpy as np

sys.path.insert(0, "/opt/trn_rl_repo")

import ml_dtypes  # noqa: E402

BF16 = ml_dtypes.bfloat16

ZONE_BLKS = (16, 16, 17)  # AG zone sizes in blocks (49 total)


@dataclass
class GCNConfig:
    n_nodes: int = 50000
    n_edges: int = 600000
    feat: int = 128
    n_layers: int = 7
    n_cores: int = 8
    n_swdge_queues: int = 4  # parallel SWDGE drain queues (1..4)
    dz: int = 13  # block positions to defer each block's zone-2 gather call
    ag_delay_blocks: int = 4  # blocks between bounce DMA and AG trigger
    nreg: int = 24  # Pool registers used for batched num_idxs loads


def _plan(cfg: GCNConfig, edge_index: np.ndarray):
    """Host graph prep: node->(core,block,slot) assignment, per-block sorted
    edge lists split by AG zone, padding, and all static counts."""
    import heapq

    N, C = cfg.n_nodes, cfg.n_cores
    nloc = N // C
    nb = (nloc + 127) // 128
    nlocp = nb * 128
    ntab = nlocp * C
    nblocks = C * nb
    ZB = ZONE_BLKS
    assert sum(ZB) == nb
    zb_bnd = [0, ZB[0], ZB[0] + ZB[1], nb]  # zone boundaries in blocks
    zrows = [z * 128 for z in ZB]  # local rows per zone
    tab_base = [0]
    for z in range(1, 3):
        tab_base.append(tab_base[-1] + C * zrows[z - 1])
    assert tab_base[2] + C * zrows[2] == ntab

    src = np.asarray(edge_index[0], dtype=np.int64)
    dst = np.asarray(edge_index[1], dtype=np.int64)
    deg = np.bincount(dst, minlength=N).astype(np.int64) + 1  # + self loop
    dis = (1.0 / np.sqrt(deg.astype(np.float64))).astype(np.float32)

    def lpt_assign(key1, key2=None):
        """Greedy assignment minimizing per-block weighted load."""
        if key2 is None:
            key2 = np.zeros_like(key1)
        order_ = np.argsort(-(key1 + key2), kind="stable")
        rows = np.empty(N, dtype=np.int64)
        heap_ = [(0.0, b) for b in range(nblocks)]
        heapq.heapify(heap_)
        f1 = np.zeros(nblocks, dtype=np.int64)
        f2 = np.zeros(nblocks, dtype=np.int64)
        cnt_ = np.zeros(nblocks, dtype=np.int64)
        s1 = 1.0 / max(1.0, key1.sum() / nblocks)
        s2 = 1.0 / max(1.0, key2.sum() / nblocks) if key2.any() else 0.0
        for n in order_:
            while True:
                f, b = heapq.heappop(heap_)
                if cnt_[b] < 128:
                    break
            rows[n] = b * 128 + cnt_[b]
            cnt_[b] += 1
            f1[b] += key1[n]
            f2[b] += key2[n]
            heapq.heappush(heap_, (f1[b] * s1 + f2[b] * s2, b))
        return rows

    def compute_tabrow(node_row):
        """Zone-major table row numbering (zone z = blocks zb_bnd[z]..[z+1] of
        every core back-to-back)."""
        cc = node_row // nlocp
        jj = node_row % nlocp
        tabrow = np.empty_like(node_row)
        for z in range(3):
            lo, hi = zb_bnd[z] * 128, zb_bnd[z + 1] * 128
            m = (jj >= lo) & (jj < hi)
            tabrow[m] = tab_base[z] + cc[m] * zrows[z] + (jj[m] - lo)
        return tabrow

    def zone_of(tabrow_vals):
        return np.digitize(tabrow_vals, [tab_base[1], tab_base[2]])

    node_row = lpt_assign(deg)
    # zone-aware refinement pass: balance (zone01_in, zone2_in) jointly so the
    # per-block ceil(cnt/128) chunk counts stay near the mean
    tabrow = compute_tabrow(node_row)
    ez = zone_of(tabrow[src])
    z01_in = np.bincount(dst[ez < 2], minlength=N)
    z2_in = np.bincount(dst[ez == 2], minlength=N)
    node_row = lpt_assign(z01_in, z2_in)

    # Relabel blocks within each core and zone so block index j has similar
    # counts on every core (the BIR is SPMD-shared, so per-j chunk counts are
    # static = max over cores).
    tabrow = compute_tabrow(node_row)
    ez = zone_of(tabrow[src])
    drow0 = node_row[dst]
    eblk0 = drow0 // 128
    z01_cnt0 = np.bincount(eblk0[ez < 2], minlength=nblocks)
    z2_cnt0 = np.bincount(eblk0[ez == 2], minlength=nblocks)
    blk_perm = np.empty(nblocks, dtype=np.int64)  # old block -> new block
    for c in range(C):
        for z in range(3):
            ids = np.arange(c * nb + zb_bnd[z], c * nb + zb_bnd[z + 1])
            order_ = ids[np.lexsort((-z2_cnt0[ids], -z01_cnt0[ids]))]
            blk_perm[order_] = ids
    node_row = blk_perm[node_row // 128] * 128 + node_row % 128

    # Final routing tables.
    tabrow = compute_tabrow(node_row)
    srow = tabrow[src]
    ez = zone_of(srow)
    drow = node_row[dst]
    eblk = drow // 128
    eslot = drow % 128
    # sort edges by (dst block, zone, src row)
    o = np.lexsort((srow, ez, eblk))
    srow_s, eblk_s, eslot_s, ez_s = srow[o], eblk[o], eslot[o], ez[o]
    starts = np.searchsorted(eblk_s, np.arange(nblocks + 1))
    zcnt = np.zeros((C, nb, 3), dtype=np.int64)
    for b in range(nblocks):
        s, e = starts[b], starts[b + 1]
        for z in range(3):
            zcnt[b // nb, b % nb, z] = int(np.count_nonzero(ez_s[s:e] == z))

    # Gather calls: zones 0+1 merged into one "lo" call per block (both AGs
    # complete before the next layer's gathers start) + one zone-2 call.
    locnt = zcnt[:, :, 0] + zcnt[:, :, 1]  # [C, nb]
    z2cnt = zcnt[:, :, 2]
    # Static per-(j, call) chunk counts (max over cores).
    nch_lo_j = np.maximum(1, (locnt.max(axis=0) + 127) // 128)
    nch_z2_j = np.maximum(1, (z2cnt.max(axis=0) + 127) // 128)
    nch_j = nch_lo_j + nch_z2_j
    nch_max = int(nch_j.max())
    seg_off = np.concatenate([[0], np.cumsum(nch_j)]).astype(np.int64)
    nsegc = int(seg_off[-1])
    t16 = nsegc * 8  # int16 idx columns (8 cols of 16-wrap per chunk)

    # Per-core packed idx (int16 relative to the call's table base, valid
    # entries first then -1 padding: the gather skips trailing negatives and
    # num_idxs_reg = count of valid ones) and segid arrays (same per-core
    # packed order; -1 seg padding nullifies unwritten/stale tile rows).
    idx_all = np.zeros((C, 128, t16), dtype=np.int16)
    seg_all = np.full((C, 128, nsegc), -1.0, dtype=BF16)

    def pack_idx(vals, n_slots):
        a = np.full(n_slots, -1, dtype=np.int16)
        a[: len(vals)] = vals
        return a.reshape(n_slots // 16, 16).T  # [16, n16]

    for c in range(C):
        for j in range(nb):
            b = c * nb + j
            s, e = starts[b], starts[b + 1]
            nlo = int(locnt[c, j])
            parts = (
                (0, srow_s[s : s + nlo], eslot_s[s : s + nlo], int(nch_lo_j[j])),
                (
                    int(nch_lo_j[j]),
                    srow_s[s + nlo : e] - tab_base[2],
                    eslot_s[s + nlo : e],
                    int(nch_z2_j[j]),
                ),
            )
            for zoff, rows, segs, nch in parts:
                col0 = (int(seg_off[j]) + zoff) * 8
                w16 = pack_idx(rows.astype(np.int16), nch * 128)
                idx_all[c, :, col0 : col0 + nch * 8] = np.tile(w16, (8, 1))
                segs = segs.astype(np.float32)
                for ch in range(math.ceil(len(segs) / 128)):
                    chunk = segs[ch * 128 : (ch + 1) * 128]
                    seg_all[c, : len(chunk), int(seg_off[j]) + zoff + ch] = (
                        chunk.astype(BF16)
                    )

    # Per-layer gather call order (positions k: lo(k), z2(k - DZ)) with
    # greedy queue balancing: call i is locked to queue i % NQ by the DMASW
    # sem-lane rotation, so order each position's calls to balance per-queue
    # desc-gen loads (mean actual counts).
    DZ = cfg.dz
    NQ = cfg.n_swdge_queues
    qload = np.zeros(NQ)
    call_list = []  # (kind, block): kind 0 = lo, 1 = zone2
    i = 0
    for k in range(nb + DZ):
        ents = []
        if k < nb:
            ents.append((0, k, float(locnt[:, k].mean())))
        if k >= DZ:
            ents.append((1, k - DZ, float(z2cnt[:, k - DZ].mean())))
        slots = [(i + t) % NQ for t in range(len(ents))]
        by_load = sorted(range(len(slots)), key=lambda t: qload[slots[t]])
        by_size = sorted(range(len(ents)), key=lambda t: -ents[t][2])
        placed = [None] * len(ents)
        for sl, en in zip(by_load, by_size):
            placed[sl] = ents[en]
        for t, (kind, j, wch) in enumerate(placed):
            qload[slots[t]] += wch
        call_list.extend((kind, j) for kind, j, _ in placed)
        i += len(ents)
    ncalls = len(call_list)
    cnts = np.zeros((C, ncalls), dtype=np.int32)
    for i, (kind, j) in enumerate(call_list):
        cnts[:, i] = locnt[:, j] if kind == 0 else z2cnt[:, j]

    # dis by table row (pads -> 0).
    dis_row = np.zeros(ntab, dtype=np.float32)
    dis_row[node_row] = dis
    return dict(
        nloc=nloc,
        nb=nb,
        nlocp=nlocp,
        ntab=ntab,
        zb_bnd=zb_bnd,
        zrows=zrows,
        tab_base=tab_base,
        nch_lo_j=nch_lo_j,
        nch_z2_j=nch_z2_j,
        nch_j=nch_j,
        nch_max=nch_max,
        seg_off=seg_off,
        t16=t16,
        nsegc=nsegc,
        call_list=call_list,
        ncalls=ncalls,
        cnts=cnts,
        node_row=node_row,
        dis_row=dis_row,
        idx_all=idx_all,
        seg_all=seg_all,
    )


def _build(cfg: GCNConfig, plan, has_bias: bool):
    """Build the SPMD Bass program (identical across cores; per-core data
    arrives via ExternalInputs)."""
    import concourse.bacc as bacc
    import concourse.tile as tile
    from concourse import mybir
    from concourse.instruction_name_ordered_set import InstructionNameOrderedSet

    dt = mybir.dt
    F, L, C = cfg.feat, cfg.n_layers, cfg.n_cores
    nb, nlocp, ntab = plan["nb"], plan["nlocp"], plan["ntab"]
    zb_bnd, zrows, tab_base = plan["zb_bnd"], plan["zrows"], plan["tab_base"]
    nch_lo_j, nch_z2_j = plan["nch_lo_j"], plan["nch_z2_j"]
    nch_max = plan["nch_max"]
    seg_off, nsegc, t16 = plan["seg_off"], plan["nsegc"], plan["t16"]
    call_list, ncalls = plan["call_list"], plan["ncalls"]
    DZ = cfg.dz
    NREG = cfg.nreg
    NGB = 17  # persistent gather buffers (round-robin over blocks;
    # coprime-ish with nb=49 so the next layer's first blocks never wait on
    # the previous layer's last consumes)

    nc = bacc.Bacc(
        "TRN2",
        target_bir_lowering=False,
        debug=False,
        num_devices=C,
        num_swdge_queues=cfg.n_swdge_queues,
    )
    RG = [list(range(C))]

    xT_d = nc.dram_tensor("xT", [F, nlocp], dt.bfloat16, kind="ExternalInput")
    W_d = nc.dram_tensor("Wb", [L, F, F], dt.bfloat16, kind="ExternalInput")
    idx_d = nc.dram_tensor("idx", [128, t16], dt.int16, kind="ExternalInput")
    seg_d = nc.dram_tensor("seg", [128, nsegc], dt.bfloat16, kind="ExternalInput")
    cnts_d = nc.dram_tensor("cnts", [1, ncalls], dt.int32, kind="ExternalInput")
    discol_d = nc.dram_tensor("discol", [128, nb], dt.float32, kind="ExternalInput")
    discol2_d = nc.dram_tensor("discol2", [128, nb], dt.float32, kind="ExternalInput")
    disrow_d = nc.dram_tensor("disrow", [1, nlocp], dt.float32, kind="ExternalInput")
    iota_d = nc.dram_tensor("iota", [128, 128], dt.bfloat16, kind="ExternalInput")
    ident_d = nc.dram_tensor("ident", [128, 128], dt.bfloat16, kind="ExternalInput")
    linw_d = nc.dram_tensor("linw", [F, 1], dt.bfloat16, kind="ExternalInput")
    linb_d = nc.dram_tensor("linb", [1, 1], dt.float32, kind="ExternalInput")
    if has_bias:
        brow_d = nc.dram_tensor("brow", [128, L * F], dt.bfloat16, kind="ExternalInput")
        invdis_d = nc.dram_tensor(
            "invdis", [128, nlocp], dt.bfloat16, kind="ExternalInput"
        )
    out_d = nc.dram_tensor("out", [1, nlocp], dt.float32, kind="ExternalOutput")

    bounces = [nc.dram_tensor(f"bounce{i}", [nlocp, F], dt.bfloat16) for i in range(2)]
    tables = [
        nc.dram_tensor(f"table{i}", [ntab, F], dt.bfloat16, addr_space="Shared")
        for i in range(2)
    ]
    ag_bnds = [0, zb_bnd[1] * 128, zb_bnd[2] * 128, nlocp]

    cregs = [nc.alloc_register(mybir.EngineType.Pool, f"cnt{i}") for i in range(NREG)]

    with tile.TileContext(nc) as tc:
        with (
            tc.tile_pool(name="const", bufs=1) as const,
            tc.tile_pool(name="spool", bufs=6) as spool,
            tc.tile_pool(name="tpool", bufs=2) as tpool,
            tc.tile_pool(name="psA", bufs=3, space="PSUM") as psA,
            tc.tile_pool(name="psO", bufs=4, space="PSUM") as psO,
            tc.tile_pool(name="psL", bufs=1, space="PSUM") as psL,
        ):
            # ---- persistent tiles + one-time loads
            h0 = const.tile([F, nlocp], dt.bfloat16, tag="h0")
            h1 = const.tile([F, nlocp], dt.bfloat16, tag="h1")
            zbufs = [
                const.tile([128, nb * F], dt.bfloat16, tag=f"zbuf{i}", name=f"zbuf{i}")
                for i in range(2)
            ]
            W_sb = const.tile([F, L * F], dt.bfloat16, tag="W")
            idx_sb = const.tile([128, t16], dt.int16, tag="idx")
            seg_sb = const.tile([128, nsegc], dt.bfloat16, tag="seg")
            cnts_sb = const.tile([1, ncalls], dt.int32, tag="cnts")
            discol = const.tile([128, nb], dt.float32, tag="discol")
            discol2 = const.tile([128, nb], dt.float32, tag="discol2")
            disrow = const.tile([1, nlocp], dt.float32, tag="disrow")
            iota = const.tile([128, 128], dt.bfloat16, tag="iota")
            ident = const.tile([128, 128], dt.bfloat16, tag="ident")
            linw = const.tile([F, 1], dt.bfloat16, tag="linw")
            linb = const.tile([1, 1], dt.float32, tag="linb")
            orow = const.tile([1, nlocp], dt.float32, tag="orow")
            if has_bias:
                brow = const.tile([128, L * F], dt.bfloat16, tag="brow")
                invdis = const.tile([128, nlocp], dt.bfloat16, tag="invdis")
            # Persistent gather buffers, zeroed once: tail chunks beyond a
            # core's actual count keep stale-but-finite data (nullified by
            # the seg=-1 zero columns in the one-hot matmul); the memset
            # guarantees the very first reads are finite too.
            g_bufs = [
                const.tile([128, nch_max, F], dt.bfloat16, tag=f"gb{i}", name=f"gb{i}")
                for i in range(NGB)
            ]
            for gb in g_bufs:
                nc.vector.memset(gb[:], 0.0)

            # h0 loaded per AG zone so stage A of early blocks starts before
            # the full input lands
            for z in range(3):
                zsl = slice(zb_bnd[z] * 128, zb_bnd[z + 1] * 128)
                nc.sync.dma_start(out=h0[:, zsl], in_=xT_d[:, zsl])
            nc.sync.dma_start(
                out=W_sb[:].rearrange("p (l f) -> p l f", f=F),
                in_=W_d[:].rearrange("l p f -> p l f"),
            )
            nc.sync.dma_start(out=idx_sb[:], in_=idx_d[:])
            nc.sync.dma_start(out=seg_sb[:], in_=seg_d[:])
            nc.sync.dma_start(out=cnts_sb[:], in_=cnts_d[:])
            nc.sync.dma_start(out=discol[:], in_=discol_d[:])
            nc.sync.dma_start(out=discol2[:], in_=discol2_d[:])
            nc.sync.dma_start(out=disrow[:], in_=disrow_d[:])
            nc.sync.dma_start(out=iota[:], in_=iota_d[:])
            nc.sync.dma_start(out=ident[:], in_=ident_d[:])
            nc.sync.dma_start(out=linw[:], in_=linw_d[:])
            nc.sync.dma_start(out=linb[:], in_=linb_d[:])
            if has_bias:
                nc.sync.dma_start(out=brow[:], in_=brow_d[:])
                nc.sync.dma_start(out=invdis[:], in_=invdis_d[:])

            hs = [h0, h1]

            def stage_a(l, j):
                """zbuf_l[:, block j] = dscale * (h_in_l[:, block j] @ W_l).

                dscale = dis for layer 0 (h0 = x) and dis^2 for layers >= 1
                (h stores unscaled u = relu(segsum); Z'' = dis^2 * (u@W))."""
                h_in = hs[l % 2]
                zbuf = zbufs[l % 2]
                dtile = discol if l == 0 else discol2
                jsl = slice(j * 128, (j + 1) * 128)
                zp = psA.tile([128, F], dt.float32, tag="zp")
                nc.tensor.matmul(
                    out=zp[:],
                    lhsT=h_in[:, jsl],
                    rhs=W_sb[:, l * F : (l + 1) * F],
                    start=True,
                    stop=True,
                )
                nc.vector.tensor_scalar_mul(
                    out=zbuf[:, jsl], in0=zp[:], scalar1=dtile[:, j : j + 1]
                )

            def ag_slice(l, s):
                """Bounce zone s of layer l's Z'' table (SBUF -> DRAM)."""
                zbuf = zbufs[l % 2]
                bounce = bounces[l % 2]
                lo_b, hi_b = ag_bnds[s], ag_bnds[s + 1]
                nc.sync.dma_start(
                    out=bounce[lo_b:hi_b, :].rearrange("(b p) f -> p b f", p=128),
                    in_=zbuf[:, lo_b * F // 128 : hi_b * F // 128].rearrange(
                        "p (b f) -> p b f", f=F
                    ),
                )
                return (l, s)

            def ag_trigger(l, s):
                bounce = bounces[l % 2]
                table = tables[l % 2]
                lo_b, hi_b = ag_bnds[s], ag_bnds[s + 1]
                sz = hi_b - lo_b
                toff = C * lo_b
                return nc.gpsimd.collective_compute(
                    "AllGather",
                    mybir.AluOpType.bypass,
                    replica_groups=RG,
                    ins=[bounce[lo_b:hi_b, :]],
                    outs=[table[toff : toff + C * sz, :]],
                )

            # GpSimd program-order chain: the 8 DMASW sem lanes rotate in
            # scheduled order and each lane is locked to one SWDGE queue, so
            # the gather queue pattern must stay periodic in that order (the
            # engine dispatches serially anyway).  Count reg loads and AG
            # triggers are chained too so the scheduler keeps them in place.
            prev_gather = None

            def _chain(inst, prev):
                if prev is not None:
                    dep = InstructionNameOrderedSet()
                    dep.add(prev.ins.name)
                    inst.ins.add_nosync_dependencies_from(dep)
                return inst

            # ---- prologue: stage A of layer 0 + its zone AllGathers
            for j in range(nb):
                stage_a(0, j)
                for z in range(3):
                    if j == zb_bnd[z + 1] - 1:
                        ag_slice(0, z)
                        prev_gather = _chain(ag_trigger(0, z), prev_gather)

            gctr = [0]

            def gather_call(tile_g, table, j, kind, reg):
                """One SWDGE gather call for block j: kind 0 = zones 0+1
                merged ("lo"), kind 1 = zone 2."""
                if kind == 0:
                    cs, w = 0, int(nch_lo_j[j])
                    tlo, thi = 0, tab_base[2]
                else:
                    cs, w = int(nch_lo_j[j]), int(nch_z2_j[j])
                    tlo, thi = tab_base[2], ntab
                o16 = (int(seg_off[j]) + cs) * 8
                ginst = nc.gpsimd.dma_gather(
                    tile_g[:, cs : cs + w, :],
                    table[tlo:thi, :],
                    idx_sb[:, o16 : o16 + w * 8],
                    w * 128,
                    reg,
                    F,
                    elem_step=F,
                    single_packet=False,
                    queue_num=gctr[0] % cfg.n_swdge_queues,
                )
                gctr[0] += 1
                return ginst

            carry_ag = None  # z2 AG trigger carried into the next layer
            for l in range(L):
                h_out = hs[(l + 1) % 2]
                zbuf = zbufs[l % 2]
                table = tables[l % 2]
                pend_ag = []  # delayed AG triggers (let bounce DMA land first)
                ci = 0  # call index within the layer
                for k in range(nb + DZ):
                    npos = (1 if k < nb else 0) + (1 if k >= DZ else 0)
                    for _ in range(npos):
                        if ci % NREG == 0:
                            nload = min(NREG, ncalls - ci)
                            linst = nc.reg_load(
                                cregs[:nload], cnts_sb[0:1, ci : ci + nload]
                            )
                            prev_gather = _chain(linst, prev_gather)
                        kind, j = call_list[ci]
                        prev_gather = _chain(
                            gather_call(
                                g_bufs[j % NGB], table, j, kind, cregs[ci % NREG]
                            ),
                            prev_gather,
                        )
                        ci += 1
                    if k == 2 and carry_ag is not None:
                        # the previous layer's z2 trigger: chained here (its
                        # bounce depends on the last consume, so chaining it
                        # at the layer boundary would stall the engine; its AG
                        # is only needed by this layer's z2 calls at pos DZ)
                        prev_gather = _chain(ag_trigger(*carry_ag), prev_gather)
                        carry_ag = None
                    if k < DZ:
                        continue
                    b = k - DZ
                    g = g_bufs[b % NGB]
                    while pend_ag and b >= pend_ag[0][2]:
                        # pin the AG trigger here in the GpSimd chain (the
                        # scheduler would otherwise sink it to layer end)
                        ent = pend_ag.pop(0)
                        prev_gather = _chain(ag_trigger(ent[0], ent[1]), prev_gather)
                    jsl = slice(b * 128, (b + 1) * 128)
                    nch = int(plan["nch_j"][b])
                    so = int(seg_off[b])
                    S = spool.tile([128, nch_max * 128], dt.bfloat16, tag="S")
                    nc.vector.tensor_tensor(
                        out=S[:, : nch * 128].rearrange("p (c f) -> p c f", f=128),
                        in0=iota[:].unsqueeze(1).to_broadcast([128, nch, 128]),
                        in1=seg_sb[:, so : so + nch]
                        .unsqueeze(2)
                        .to_broadcast([128, nch, 128]),
                        op=mybir.AluOpType.is_equal,
                    )
                    O = psO.tile([F, 128], dt.float32, tag="O")
                    nc.tensor.matmul(
                        out=O[:], lhsT=zbuf[:, jsl], rhs=ident[:], start=True,
                        stop=False,
                    )
                    for ch in range(nch):
                        nc.tensor.matmul(
                            out=O[:],
                            lhsT=g[:, ch, :],
                            rhs=S[:, ch * 128 : (ch + 1) * 128],
                            start=False,
                            stop=(ch == nch - 1 and not has_bias),
                        )
                    if has_bias:
                        # O += outer(b_l, 1/dis): exact bias support since
                        # relu(dis*x + b) = dis*relu(x + b/dis)
                        nc.tensor.matmul(
                            out=O[:],
                            lhsT=brow[:, l * F : (l + 1) * F],
                            rhs=invdis[:, jsl],
                            start=False,
                            stop=True,
                        )
                    nc.scalar.activation(
                        out=h_out[:, jsl],
                        in_=O[:],
                        func=mybir.ActivationFunctionType.Relu,
                        scale=1.0,
                    )
                    if l + 1 < L:
                        stage_a(l + 1, b)
                        for z in range(3):
                            if b == zb_bnd[z + 1] - 1:
                                ag_slice(l + 1, z)
                                pend_ag.append(
                                    (l + 1, z, b + cfg.ag_delay_blocks)
                                )
                for ent in pend_ag:
                    if ent[1] == 2:
                        carry_ag = (ent[0], ent[1])
                    else:
                        prev_gather = _chain(
                            ag_trigger(ent[0], ent[1]), prev_gather
                        )

            # ---- final linear readout: out = dis * (linw.T @ u7) + lin_b
            h_fin = hs[L % 2]
            for k in range(0, nlocp, 512):
                w = min(512, nlocp - k)
                op = psL.tile([1, 512], dt.float32, tag="op")
                nc.tensor.matmul(
                    out=op[:, :w], lhsT=linw[:], rhs=h_fin[:, k : k + w],
                    start=True, stop=True,
                )
                tr = tpool.tile([1, 512], dt.float32, tag="tr")
                nc.vector.tensor_tensor(
                    out=tr[:, :w], in0=op[:, :w], in1=disrow[:, k : k + w],
                    op=mybir.AluOpType.mult,
                )
                nc.scalar.activation(
                    out=orow[:, k : k + w],
                    in_=tr[:, :w],
                    func=mybir.ActivationFunctionType.Identity,
                    bias=linb[:],
                    scale=1.0,
                )
            nc.sync.dma_start(out=out_d[:], in_=orow[:])
    nc.compile()
    return nc


def _make_inputs(cfg: GCNConfig, plan, x, Ws, bs, lin_w, lin_b, has_bias):
    """Per-core in_maps from full inputs + plan."""
    C, F, L = cfg.n_cores, cfg.feat, cfg.n_layers
    nlocp, nb = plan["nlocp"], plan["nb"]
    node_row = plan["node_row"]
    dis_row = plan["dis_row"]

    x = np.asarray(x, dtype=np.float32)
    Ws = np.asarray(Ws, dtype=np.float32)
    bs = np.asarray(bs, dtype=np.float32)
    lin_w = np.asarray(lin_w, dtype=np.float32)
    lin_b = np.asarray(lin_b, dtype=np.float32)

    xrow = np.zeros((C * nlocp, F), dtype=np.float32)
    xrow[node_row] = x
    Wb = Ws.astype(BF16)
    iota = np.tile(np.arange(128, dtype=np.float32), (128, 1)).astype(BF16)
    ident = np.eye(128, dtype=np.float32).astype(BF16)
    linw = lin_w.reshape(F, 1).astype(BF16)
    linb = lin_b.reshape(1, 1).astype(np.float32)
    if has_bias:
        browf = np.zeros((128, L * F), dtype=np.float32)
        browf[0] = bs.reshape(L * F)
        brow = browf.astype(BF16)

    in_maps = []
    for c in range(C):
        rows = slice(c * nlocp, (c + 1) * nlocp)
        dloc = dis_row[rows]
        im = {
            "xT": np.ascontiguousarray(xrow[rows].T).astype(BF16),
            "Wb": Wb,
            "idx": np.ascontiguousarray(plan["idx_all"][c]),
            "seg": np.ascontiguousarray(plan["seg_all"][c]),
            "cnts": plan["cnts"][c].reshape(1, -1),
            "discol": np.ascontiguousarray(dloc.reshape(nb, 128).T),
            "discol2": np.ascontiguousarray((dloc * dloc).reshape(nb, 128).T),
            "disrow": dloc.reshape(1, nlocp),
            "iota": iota,
            "ident": ident,
            "linw": linw,
            "linb": linb,
        }
        if has_bias:
            invd = np.where(dloc > 0, 1.0 / np.maximum(dloc, 1e-30), 0.0)
            invf = np.zeros((128, nlocp), dtype=np.float32)
            invf[0] = invd
            im["brow"] = brow
            im["invdis"] = invf.astype(BF16)
        in_maps.append(im)
    return in_maps


def _reassemble(cfg: GCNConfig, plan, outs):
    nlocp = plan["nlocp"]
    node_row = plan["node_row"]
    full = np.zeros(cfg.n_cores * nlocp, dtype=np.float32)
    for c, o in enumerate(outs):
        full[c * nlocp : (c + 1) * nlocp] = o["out"].reshape(-1)
    return full[node_row]


def kernel(**inputs) -> np.ndarray:
    cfg = GCNConfig()
    return _kernel_impl(cfg, inputs, mode=os.environ.get("GCN_MODE", "hw"))


def _kernel_impl(cfg: GCNConfig, inputs, mode="hw", trace=False):
    x = np.asarray(inputs["x"])
    edge_index = np.asarray(inputs["edge_index"])
    bs = np.asarray(inputs["bs"])
    has_bias = bool(np.any(bs != 0))
    plan = _plan(cfg, edge_index)
    nc = _build(cfg, plan, has_bias)
    in_maps = _make_inputs(
        cfg, plan, x, inputs["Ws"], bs, inputs["lin_w"], inputs["lin_b"], has_bias
    )
    if mode == "sim":
        from concourse import bass_interp

        sim = bass_interp.MultiCoreSim(nc, cfg.n_cores)
        for c in range(cfg.n_cores):
            for k, v in in_maps[c].items():
                sim.cores[c].tensor(k)[:] = v
        sim.simulate()
        outs = [
            {"out": np.asarray(sim.cores[c].mem_tensor("out"))}
            for c in range(cfg.n_cores)
        ]
        result = _reassemble(cfg, plan, outs)
        return result.astype(np.float32)
    else:
        from concourse.bass_utils import run_bass_kernel_spmd

        res = run_bass_kernel_spmd(
            nc, in_maps, core_ids=list(range(cfg.n_cores)), trace=trace
        )
        out = _reassemble(cfg, plan, res.results)
        if trace:
            return out.astype(np.float32), res
        return out.astype(np.float32)


if __name__ == "__main__":
    pass
